# revision 17
# baseline (speedup 1.0000x reference)
"""Trainium2 Bass kernel: 3-layer GraphSAGE (mean aggr) + 3 classification heads.

Strategy (8 NeuronCores, SPMD, node-partitioned):
  - Nodes are sharded by contiguous range across the 8 cores (6250 each).
  - Edges are sorted by dst on the host and bucketed into per-core,
    per-128-node-block chunk structures (CSR-like, padded position-wise so
    all 8 cores share one compiled graph). Because dma_gather indices are
    int16, each block's edges are split by src < 25000 (lo) / >= 25000 (hi)
    and gathered from the corresponding half of the node table.
  - Aggregation (segment-mean) runs on the TensorEngine: gather h[src] rows
    chunk-major with dma_gather (one per super-block of 3 node blocks and
    table half), build a selection matrix S[e,n] = (local_dst[e]==n) *
    inv_deg[e] with one fused DVE op per 128-edge chunk, and accumulate
    matmul(lhsT=gathered_chunk, rhs=S) into PSUM -> mean^T blocks.
  - Dense parts (mean @ Wl + h @ Wr + b, ReLU) are feature-major matmuls;
    a per-layer AllGather replicates h for the next layer's gather.
  - The three heads share one aggregation: project y3 = h3 @ [Wl_age|sex|eth]
    (128->28, padded to 64 cols for the 256B-row gather constraint) BEFORE
    aggregating.
"""

import os
import sys

import numpy as np

for _p in ("/opt/trn_rl_repo", "/root/.axon_site/_ro/trn_rl_repo"):
    if os.path.isdir(_p) and _p not in sys.path:
        sys.path.insert(0, _p)

import concourse.bass as bass
import concourse.mybir as mybir
import concourse.tile as tile
from concourse import bacc
from concourse.bass_utils import run_bass_kernel_spmd
from concourse.masks import make_identity

F32 = mybir.dt.float32
I16 = mybir.dt.int16
P = 128

# Problem constants (hardcoded per spec)
N_NODES = 50000
N_EDGES = 800000
IN_CH = 64
HID = 128
OUT_AGE, OUT_SEX, OUT_ETH = 21, 2, 5
HOUT = OUT_AGE + OUT_SEX + OUT_ETH  # 28
HOUTP = 64  # head gather width padded to 256B rows
N_CORES = 8
SB = 3  # node blocks per super-block (per dma_gather pair)


def real_cfg():
    npc = N_NODES // N_CORES
    nb = (npc + P - 1) // P
    return dict(
        n_nodes=N_NODES,
        n_cores=N_CORES,
        npc=npc,
        nb=nb,
        last_bs=npc - (nb - 1) * P,
        in_ch=IN_CH,
        hid=HID,
        hout=HOUT,
        houtp=HOUTP,
        ch=512,
        sb=SB,
    )


GCAP = 8  # chunks per dma_gather (1024 idx = SWDGE descriptor-ring capacity)


class Plan:
    """Compile-time chunk layout shared by host fill and device emit.

    Global chunk columns: lo chunks of all blocks in block order, then hi
    chunks of all blocks. Each half-region is aggregated in its own PSUM
    pass (hi adds into meanT). Gathers are runs of <= GCAP chunks within a
    region, crossing block boundaries.
    """

    def __init__(self, Klo, Khi, sb=None):
        nb = len(Klo)
        self.Klo, self.Khi = list(Klo), list(Khi)
        self.lo_range = {}
        self.hi_range = {}
        cur = 0
        for b in range(nb):
            self.lo_range[b] = (cur, cur + Klo[b])
            cur += Klo[b]
        self.tklo = cur
        for b in range(nb):
            self.hi_range[b] = (cur, cur + Khi[b])
            cur += Khi[b]
        self.tk = cur
        # gathers: (col0, kh, is_lo) runs of <= GCAP chunks
        self.gathers = []
        for (r0, r1, lo) in ((0, self.tklo, True), (self.tklo, self.tk, False)):
            for c in range(r0, r1, GCAP):
                self.gathers.append((c, min(GCAP, r1 - c), lo))


def prepare_edges(edge_src, edge_dst, cfg):
    """Returns (plan, G16, A_id, A_inv):
      G16[c]  = [128, tk*8] int16  dma_gather index stream (i%16 row, i//16
                col within each half-range; 16-row pattern tiled to 128)
      A_id[c] = [128, tk] f32  dst id local to node block (pad=300)
      A_inv[c]= [128, tk] f32  1/max(deg[dst],1) per edge (pad=0)
    """
    npc, nb, nc_ = cfg["npc"], cfg["nb"], cfg["n_cores"]
    n = cfg["n_nodes"]
    half = n // 2
    order = np.argsort(edge_dst, kind="stable")
    src = edge_src[order].astype(np.int64)
    dst = edge_dst[order].astype(np.int64)
    deg = np.bincount(dst, minlength=n)
    inv = (1.0 / np.maximum(deg, 1)).astype(np.float32)
    inv_e = inv[dst]

    core = dst // npc
    dst_local = dst - core * npc
    blk = dst_local >> 7
    lid = (dst_local & 127).astype(np.float32)
    islo = src < half
    gb = core * nb + blk  # sorted

    nlo = np.zeros((nc_, nb), np.int64)
    nhi = np.zeros((nc_, nb), np.int64)
    bounds = np.searchsorted(gb, np.arange(nc_ * nb + 1))
    for c in range(nc_):
        for b in range(nb):
            s, e = bounds[c * nb + b], bounds[c * nb + b + 1]
            nlo[c, b] = islo[s:e].sum()
            nhi[c, b] = (e - s) - nlo[c, b]
    Klo = np.maximum(1, -(-nlo.max(axis=0) // P)).astype(np.int64)
    Khi = -(-nhi.max(axis=0) // P).astype(np.int64)
    plan = Plan([int(k) for k in Klo], [int(k) for k in Khi])
    tk = plan.tk

    # chunk-major staging arrays: [tk, 128]
    idx_all = np.zeros((nc_, tk, P), np.int16)
    id_all = np.full((nc_, tk, P), 300.0, np.float32)
    inv_all = np.zeros((nc_, tk, P), np.float32)
    for c in range(nc_):
        for b in range(nb):
            s, e = bounds[c * nb + b], bounds[c * nb + b + 1]
            if e == s:
                continue
            m_lo = islo[s:e]
            for want_lo in (True, False):
                sel = m_lo if want_lo else ~m_lo
                cnt = int(sel.sum())
                if cnt == 0:
                    continue
                srcs = src[s:e][sel] - (0 if want_lo else half)
                k0 = (plan.lo_range if want_lo else plan.hi_range)[b][0]
                j = np.arange(cnt)
                kk = k0 + (j >> 7)
                pp = j & 127
                idx_all[c, kk, pp] = srcs.astype(np.int16)
                id_all[c, kk, pp] = lid[s:e][sel]
                inv_all[c, kk, pp] = inv_e[s:e][sel]

    # A_id / A_inv: slot (partition, chunk)
    A_id = np.ascontiguousarray(id_all.transpose(0, 2, 1))
    A_inv = np.ascontiguousarray(inv_all.transpose(0, 2, 1))

    # G16: slot (chunk k, p) lives at row p%16, col k*8 + p//16
    # (chunk-major stream wrapped into 16 rows; identical for any gather
    # grouping since 128 % 16 == 0)
    G16_16 = (
        idx_all.reshape(nc_, tk, 8, 16)  # [c, k, p//16, p%16]
        .transpose(0, 3, 1, 2)           # [c, p%16, k, p//16]
        .reshape(nc_, 16, tk * 8)
    )
    G16 = np.ascontiguousarray(np.tile(G16_16, (1, 8, 1)))
    return plan, G16, A_id, A_inv


# ---------------------------------------------------------------------------
# Device graph
# ---------------------------------------------------------------------------

def build_graph(cfg, plan):
    npc, nb, last_bs = cfg["npc"], cfg["nb"], cfg["last_bs"]
    in_ch, hid, hout, houtp = cfg["in_ch"], cfg["hid"], cfg["hout"], cfg["houtp"]
    n_nodes, n_cores, CH = cfg["n_nodes"], cfg["n_cores"], cfg["ch"]
    half = n_nodes // 2
    tk = plan.tk
    RG = [list(range(n_cores))]
    nchunks = (npc + CH - 1) // CH

    nc = bacc.Bacc("TRN2", target_bir_lowering=False, debug=False,
                   num_swdge_queues=2)

    # ---- I/O ----
    x_full_d = nc.dram_tensor("x_full", [n_nodes, in_ch], F32, kind="ExternalInput")
    xT_d = nc.dram_tensor("xT", [in_ch, npc], F32, kind="ExternalInput")
    G16_d = nc.dram_tensor("G16", [P, tk * 8], I16, kind="ExternalInput")
    Aid_d = nc.dram_tensor("Aid", [P, tk], F32, kind="ExternalInput")
    Ainv_d = nc.dram_tensor("Ainv", [P, tk], F32, kind="ExternalInput")
    iota_d = nc.dram_tensor("iota", [P, P], F32, kind="ExternalInput")
    wspec = dict(
        Wl1=(in_ch, hid), Wr1=(in_ch, hid), b1=(hid, 1),
        Wl2=(hid, hid), Wr2=(hid, hid), b2=(hid, 1),
        Wl3=(hid, hid), Wr3=(hid, hid), b3=(hid, 1),
        Wlh=(hid, hout), Wrh=(hid, hout), bh=(hout, 1),
    )
    wd = {k: nc.dram_tensor(k, list(s), F32, kind="ExternalInput")
          for k, s in wspec.items()}
    outT_d = nc.dram_tensor("outT", [hout, npc], F32, kind="ExternalOutput")

    # ---- internal DRAM (collective bounce) ----
    h1_loc = nc.dram_tensor("h1_loc", [npc, hid], F32)
    h1_full = nc.dram_tensor("h1_full", [n_nodes, hid], F32, addr_space="Shared")
    h2_loc = nc.dram_tensor("h2_loc", [npc, hid], F32)
    h2_full = nc.dram_tensor("h2_full", [n_nodes, hid], F32, addr_space="Shared")
    y3_loc = nc.dram_tensor("y3_loc", [npc, houtp], F32)
    y3_full = nc.dram_tensor("y3_full", [n_nodes, houtp], F32, addr_space="Shared")

    with tile.TileContext(nc) as tc:
        with (
            tc.tile_pool(name="const", bufs=1) as constp,
            tc.tile_pool(name="edge", bufs=1) as edgep,
            tc.tile_pool(name="hT", bufs=2) as hTp,
            tc.tile_pool(name="meanT", bufs=1) as meanp,
            tc.tile_pool(name="gath", bufs=2) as gathp,
            tc.tile_pool(name="sel", bufs=4) as selp,
            tc.tile_pool(name="strm", bufs=2) as strmp,
            tc.tile_pool(name="tpo", bufs=4) as tpop,
            tc.tile_pool(name="psA", bufs=2, space="PSUM") as psA,
            tc.tile_pool(name="psL", bufs=2, space="PSUM") as psL,
            tc.tile_pool(name="psT", bufs=2, space="PSUM") as psT,
        ):
            # ---- constants ----
            iota_sb = constp.tile([P, P], F32, tag="iota", name="iota_sb")
            nc.sync.dma_start(out=iota_sb[:], in_=iota_d[:])
            ident_sb = constp.tile([P, P], F32, tag="ident", name="ident_sb")
            make_identity(nc, ident_sb[:])
            w = {}
            for k, (r, c) in wspec.items():
                w[k] = constp.tile([r, c], F32, tag=k, name=f"w_{k}")
                nc.sync.dma_start(out=w[k][:], in_=wd[k][:])
            G16_sb = edgep.tile([P, tk * 8], I16, tag="G16", name="G16_sb")
            nc.sync.dma_start(out=G16_sb[:], in_=G16_d[:])
            Aid_sb = edgep.tile([P, tk], F32, tag="Aid", name="Aid_sb")
            nc.sync.dma_start(out=Aid_sb[:], in_=Aid_d[:])
            Ainv_sb = edgep.tile([P, tk], F32, tag="Ainv", name="Ainv_sb")
            nc.sync.dma_start(out=Ainv_sb[:], in_=Ainv_d[:])

            def blk_size(b):
                return P if b < nb - 1 else last_bs

            # chunk col -> (gather id, position within gather)
            chunk2g = {}
            for gid, (col0, kh, lo) in enumerate(plan.gathers):
                for k in range(col0, col0 + kh):
                    chunk2g[k] = (gid, k - col0)

            def emit_agg(src_dram, d, lo_epilogue, hi_epilogue):
                """Segment-mean over node blocks in two passes (lo/hi table
                half). Epilogues consume each finished [d, bs] psum block."""
                gt_tiles = {}

                def get_gather(gid):
                    if gid in gt_tiles:
                        return gt_tiles[gid]
                    col0, kh, lo = plan.gathers[gid]
                    gt = gathp.tile([P, GCAP * d], F32, tag="g",
                                    name=f"g_{gid}")
                    nc.gpsimd.dma_gather(
                        out_ap=gt[:, : kh * d].rearrange(
                            "p (k d) -> p k d", k=kh),
                        in_ap=(src_dram[0:half, :] if lo
                               else src_dram[half : 2 * half, :]),
                        idxs_ap=G16_sb[:, col0 * 8 : (col0 + kh) * 8],
                        num_idxs=kh * P,
                        num_idxs_reg=kh * P,
                        elem_size=d,
                        queue_num=gid % 2,
                    )
                    gt_tiles[gid] = gt
                    return gt

                for lo in (True, False):
                    rng_map = plan.lo_range if lo else plan.hi_range
                    epi = lo_epilogue if lo else hi_epilogue
                    for b in range(nb):
                        k0, k1 = rng_map[b]
                        if k1 == k0:
                            continue
                        ps = psA.tile([P, P], F32, tag="agg", name="agg_ps")
                        for i, k in enumerate(range(k0, k1)):
                            gid, pos = chunk2g[k]
                            gt = get_gather(gid)
                            S = selp.tile([P, P], F32, tag="S", name="S_t")
                            nc.vector.scalar_tensor_tensor(
                                out=S[:],
                                in0=iota_sb[:],
                                scalar=Aid_sb[:, k : k + 1],
                                in1=Ainv_sb[:, k : k + 1].to_broadcast([P, P]),
                                op0=mybir.AluOpType.is_equal,
                                op1=mybir.AluOpType.mult,
                            )
                            nc.tensor.matmul(
                                out=ps[:d, :],
                                lhsT=gt[:, pos * d : (pos + 1) * d],
                                rhs=S[:],
                                start=(i == 0),
                                stop=(i == k1 - k0 - 1),
                            )
                        epi(b, blk_size(b), ps)

            def emit_dense(Wl, Wr, bias, rhs1, d1, rhs2, d2, do, outT, func,
                           rhs2_dram=False):
                """outT[:do,:] = func(Wl.T @ rhs1 + Wr.T @ rhs2 + bias)."""
                for j in range(nchunks):
                    c0 = j * CH
                    cw = min(CH, npc - c0)
                    if rhs2_dram:
                        r2t = strmp.tile([d2, CH], F32, tag="xc", name="xc_t")
                        nc.sync.dma_start(out=r2t[:, :cw], in_=rhs2[:, c0 : c0 + cw])
                        r2 = r2t[:d2, :cw]
                    else:
                        r2 = rhs2[:d2, c0 : c0 + cw]
                    ps = psL.tile([P, CH], F32, tag="L", name="L_ps")
                    nc.tensor.matmul(
                        out=ps[:do, :cw], lhsT=Wl[:d1, :do],
                        rhs=rhs1[:d1, c0 : c0 + cw], start=True, stop=False,
                    )
                    nc.tensor.matmul(
                        out=ps[:do, :cw], lhsT=Wr[:d2, :do],
                        rhs=r2, start=False, stop=True,
                    )
                    nc.scalar.activation(
                        out=outT[:do, c0 : c0 + cw], in_=ps[:do, :cw],
                        func=func, bias=bias[:do, :1], scale=1.0,
                    )

            def emit_to_node_major(srcT, d, loc_dram, b, bs, col0, pad_to=None):
                """Transpose srcT[:d, col0:col0+bs] -> loc_dram[b*128+..., :d]."""
                pt = psT.tile([P, P], F32, tag="T", name="T_ps")
                nc.tensor.transpose(
                    out=pt[:bs, :d], in_=srcT[:d, col0 : col0 + bs],
                    identity=ident_sb[:d, :d],
                )
                wd_ = pad_to or d
                st = tpop.tile([P, max(hid, hout, wd_)], F32, tag="tp",
                               name="tp_t")
                nc.scalar.copy(out=st[:bs, :d], in_=pt[:bs, :d])
                if wd_ > d:
                    nc.gpsimd.memset(st[:bs, d:wd_], 0.0)
                nc.sync.dma_start(
                    out=loc_dram[b * P : b * P + bs, :wd_], in_=st[:bs, :wd_]
                )

            def emit_ag(loc, full):
                nc.gpsimd.collective_compute(
                    "AllGather",
                    mybir.AluOpType.bypass,
                    ins=[loc[:]],
                    outs=[full[:]],
                    replica_groups=RG,
                )

            relu = mybir.ActivationFunctionType.Relu

            def mean_epi(meanT, d):
                def _epi(b, bs, ps):
                    nc.scalar.copy(
                        out=meanT[:d, b * P : b * P + bs], in_=ps[:d, :bs]
                    )
                return _epi

            def mean_add_epi(meanT, d):
                def _epi(b, bs, ps):
                    nc.vector.scalar_tensor_tensor(
                        out=meanT[:d, b * P : b * P + bs],
                        in0=ps[:d, :bs],
                        scalar=0.0,
                        in1=meanT[:d, b * P : b * P + bs],
                        op0=mybir.AluOpType.add,
                        op1=mybir.AluOpType.add,
                    )
                return _epi

            def agg_mean(src_dram, d, meanT):
                emit_agg(src_dram, d, mean_epi(meanT, d),
                         mean_add_epi(meanT, d))

            # ---------------- Layer 1 ----------------
            mean1 = meanp.tile([P, npc], F32, tag="mean", name="mean1")
            agg_mean(x_full_d, in_ch, mean1)
            h1T = hTp.tile([P, npc], F32, tag="hT", name="h1T")
            emit_dense(w["Wl1"], w["Wr1"], w["b1"], mean1, in_ch, xT_d, in_ch,
                       hid, h1T, relu, rhs2_dram=True)
            for b in range(nb):
                emit_to_node_major(h1T, hid, h1_loc, b, blk_size(b), b * P)
            emit_ag(h1_loc, h1_full)

            # ---------------- Layer 2 ----------------
            mean2 = meanp.tile([P, npc], F32, tag="mean", name="mean2")
            agg_mean(h1_full, hid, mean2)
            h2T = hTp.tile([P, npc], F32, tag="hT", name="h2T")
            emit_dense(w["Wl2"], w["Wr2"], w["b2"], mean2, hid, h1T, hid,
                       hid, h2T, relu)
            for b in range(nb):
                emit_to_node_major(h2T, hid, h2_loc, b, blk_size(b), b * P)
            emit_ag(h2_loc, h2_full)

            # ---------------- Layer 3 ----------------
            mean3 = meanp.tile([P, npc], F32, tag="mean", name="mean3")
            agg_mean(h2_full, hid, mean3)
            h3T = hTp.tile([P, npc], F32, tag="hT", name="h3T")
            emit_dense(w["Wl3"], w["Wr3"], w["b3"], mean3, hid, h2T, hid,
                       hid, h3T, relu)

            # ---------------- Heads ----------------
            # y3 = h3 @ Wl_heads (project 128->28 before aggregating)
            for j in range(nchunks):
                c0 = j * CH
                cw = min(CH, npc - c0)
                ps = psL.tile([P, CH], F32, tag="L", name="y3_ps")
                nc.tensor.matmul(
                    out=ps[:hout, :cw], lhsT=w["Wlh"][:hid, :hout],
                    rhs=h3T[:hid, c0 : c0 + cw], start=True, stop=True,
                )
                y3c = strmp.tile([hout, CH], F32, tag="y3c", name="y3c_t")
                nc.scalar.copy(out=y3c[:hout, :cw], in_=ps[:hout, :cw])
                for t in range((cw + P - 1) // P):
                    b = (c0 // P) + t
                    emit_to_node_major(y3c, hout, y3_loc, b,
                                       min(blk_size(b), cw - t * P), t * P,
                                       pad_to=houtp)
            emit_ag(y3_loc, y3_full)

            # head aggregation (two passes into meanY), then per-block
            # WrTerm + bias + meanY combine
            meanY = meanp.tile([P, npc], F32, tag="mean", name="meanY")
            emit_agg(y3_full, houtp, mean_epi(meanY, hout),
                     mean_add_epi(meanY, hout))
            for b in range(nb):
                bs = blk_size(b)
                psw = psL.tile([P, CH], F32, tag="L", name="wr_ps")
                nc.tensor.matmul(
                    out=psw[:hout, :bs], lhsT=w["Wrh"][:hid, :hout],
                    rhs=h3T[:hid, b * P : b * P + bs], start=True, stop=True,
                )
                ob = tpop.tile([hout, P], F32, tag="ob", name="ob_t")
                nc.vector.scalar_tensor_tensor(
                    out=ob[:hout, :bs],
                    in0=psw[:hout, :bs],
                    scalar=w["bh"][:hout, :1],
                    in1=meanY[:hout, b * P : b * P + bs],
                    op0=mybir.AluOpType.add,
                    op1=mybir.AluOpType.add,
                )
                nc.sync.dma_start(
                    out=outT_d[:, b * P : b * P + bs], in_=ob[:hout, :bs]
                )

    nc.compile()
    return nc


# ---------------------------------------------------------------------------
# Entry point
# ---------------------------------------------------------------------------

_GRAPH_CACHE = {}
_LAST_RESULTS = None


def _get_graph(cfg, plan):
    key = (tuple(sorted(cfg.items())), tuple(plan.Klo), tuple(plan.Khi))
    if key not in _GRAPH_CACHE:
        _GRAPH_CACHE[key] = build_graph(cfg, plan)
    return _GRAPH_CACHE[key]


def make_in_maps(inputs, cfg, G16, A_id, A_inv):
    x = np.ascontiguousarray(np.asarray(inputs["x"], np.float32))
    npc = cfg["npc"]
    iota = np.tile(np.arange(P, dtype=np.float32), (P, 1))
    Wlh = np.concatenate(
        [np.asarray(inputs["Wl_age"]), np.asarray(inputs["Wl_sex"]),
         np.asarray(inputs["Wl_eth"])], axis=1).astype(np.float32)
    Wrh = np.concatenate(
        [np.asarray(inputs["Wr_age"]), np.asarray(inputs["Wr_sex"]),
         np.asarray(inputs["Wr_eth"])], axis=1).astype(np.float32)
    bh = np.concatenate(
        [np.asarray(inputs["b_age"]), np.asarray(inputs["b_sex"]),
         np.asarray(inputs["b_eth"])]).astype(np.float32)
    shared = dict(
        x_full=x,
        iota=iota,
        Wl1=np.ascontiguousarray(np.asarray(inputs["Wl_1"], np.float32)),
        Wr1=np.ascontiguousarray(np.asarray(inputs["Wr_1"], np.float32)),
        b1=np.asarray(inputs["b_1"], np.float32).reshape(-1, 1),
        Wl2=np.ascontiguousarray(np.asarray(inputs["Wl_2"], np.float32)),
        Wr2=np.ascontiguousarray(np.asarray(inputs["Wr_2"], np.float32)),
        b2=np.asarray(inputs["b_2"], np.float32).reshape(-1, 1),
        Wl3=np.ascontiguousarray(np.asarray(inputs["Wl_3"], np.float32)),
        Wr3=np.ascontiguousarray(np.asarray(inputs["Wr_3"], np.float32)),
        b3=np.asarray(inputs["b_3"], np.float32).reshape(-1, 1),
        Wlh=Wlh, Wrh=Wrh, bh=bh.reshape(-1, 1),
    )
    in_maps = []
    for c in range(cfg["n_cores"]):
        m = dict(shared)
        m["xT"] = np.ascontiguousarray(x[c * npc : (c + 1) * npc].T)
        m["G16"] = G16[c]
        m["Aid"] = A_id[c]
        m["Ainv"] = A_inv[c]
        in_maps.append(m)
    return in_maps


def kernel(**inputs):
    global _LAST_RESULTS
    cfg = real_cfg()
    edge_src = np.asarray(inputs["edge_src"])
    edge_dst = np.asarray(inputs["edge_dst"])
    plan, G16, A_id, A_inv = prepare_edges(edge_src, edge_dst, cfg)
    nc = _get_graph(cfg, plan)
    in_maps = make_in_maps(inputs, cfg, G16, A_id, A_inv)
    trace = bool(os.environ.get("GNN_TRACE"))
    res = run_bass_kernel_spmd(
        nc, in_maps, list(range(cfg["n_cores"])), trace=trace
    )
    _LAST_RESULTS = res
    outT = np.concatenate(
        [res.results[c]["outT"] for c in range(cfg["n_cores"])], axis=1
    )
    out = np.ascontiguousarray(outT.T)  # [n_nodes, 28]
    return (
        out[:, :OUT_AGE],
        out[:, OUT_AGE : OUT_AGE + OUT_SEX],
        out[:, OUT_AGE + OUT_SEX :],
    )
